# revision 28
# baseline (speedup 1.0000x reference)
"""Trainium2 Bass kernel for a context-LSTM decoder.

Model (B=256, T=256, I=H=1024, 4H=4096, F=512, NC=7):
    ctx   = v @ Wc.T + (bc + bi + bh)                      # [B, 4H], const over t
    gx    = i_features @ Wi.T + ctx                        # [B, T, 4H]
    per t: gates = gx[t] + h @ Wh.T ; LSTM cell update
    out   = relu(h_T @ Wfa.T + bfa) @ Wfc.T + bfc          # [B, 7]

Strategy: pure data-parallel over batch, 32 rows per core, no collectives.
All matmul operands fp16 (fp32 PSUM accumulation), cell state c fp32.
Host pre-transposes x to [I, T, Bs] and weights to K-major so every DMA is
contiguous; the only on-device transposes are the per-step h -> h_T
(DMA xbar transpose, off the critical PE path).

Phase 1 (big GEMM) tiles M=(4t x 32b)=128 rows at full PE width; phase 2
streams Wh through the PE as the moving operand (64 matmuls of N=512 per
step), which is the per-step floor; DVE adds gx during PSUM drain and ACT
does sigmoid/tanh (one shared table set).

An fp8 DoubleRow recurrence variant (fp8_rec=True) was implemented and is
HW-correct (rel err 6.5e-3) but measured NO faster than fp16: the step is
bound by the serial drain/cell/transpose chain at ~13-14us, which the fp16
schedule already fully hides under its PE streaming; halving PE work just
exposes the chain. Kept for reference, default off.

build_lstm(n_reps=R) emits the whole computation R times back-to-back in
one NEFF so test.py can measure true HW exec time as wall(R=2)-wall(R=1),
cancelling the ~80ms axon-relay dispatch overhead that is not kernel time.
"""

import numpy as np
from contextlib import ExitStack

import concourse.bass as bass
import concourse.bacc as bacc
import concourse.mybir as mybir
import concourse.tile as tile
from concourse.bass_utils import run_bass_kernel_spmd

B, T, I = 256, 256, 1024
H = 1024
G = 4 * H
F = 512
NCLS = 7
NCORES = 8
BS = B // NCORES          # 32 batch rows per core
P = 128                   # partitions
KI = I // P               # 8 k-tiles over the input dim
KH = H // P               # 8 k-tiles over the hidden dim
NCH = G // 512            # 8 chunks of 512 gate columns
F16 = mybir.dt.float16
F32 = mybir.dt.float32
AOP = mybir.AluOpType
AFT = mybir.ActivationFunctionType

# chunk n holds gate columns [512n, 512n+512); ig=0,1 fg=2,3 cg=4,5 og=6,7.
# Emit in this order so half 0 (chunks 0,2,4,6 -> units 0:512) finishes first.
CHUNK_ORDER = [0, 2, 4, 6, 1, 3, 5, 7]


def build_lstm(t_steps: int = T, rec_steps: int | None = None,
               no_gx_mm: bool = False, gx_via_dve: bool = True,
               ph1_wr_engine: str = "gpsimd", ph1_no_write: bool = False,
               ph1_no_drain: bool = False, n_reps: int = 1,
               fp8_rec: bool = False):
    # rec_steps: number of recurrence iterations (defaults to t_steps). When
    # larger than t_steps, gx rows are reused cyclically (timing studies only).
    # n_reps: emit the whole computation n_reps times back-to-back in one NEFF
    # (identical I/O signature); R=2 minus R=1 wall-clock isolates HW exec
    # time from the fixed axon dispatch overhead.
    if rec_steps is None:
        rec_steps = t_steps
    assert t_steps % 4 == 0
    n_mtiles = t_steps * BS // P          # phase-1 M-tiles (128 bt-rows each)
    nc = bacc.Bacc("TRN2", target_bir_lowering=False, debug=False,
                   num_devices=NCORES)

    F8 = mybir.dt.float8e4
    xT = nc.declare_dram_parameter("xT", [I, t_steps, BS], F16, isOutput=False)
    vT = nc.declare_dram_parameter("vT", [I, BS], F16, isOutput=False)
    WiT = nc.declare_dram_parameter("WiT", [I, G], F16, isOutput=False)
    if fp8_rec:
        # 64*Wh.T packed for DoubleRow: [p, dk, e, g] = 64*Wh.T[256dk+128e+p, g]
        WhT8 = nc.declare_dram_parameter("WhT8", [P, KH // 2, 2, G], F8,
                                         isOutput=False)
    else:
        WhT = nc.declare_dram_parameter("WhT", [H, G], F16, isOutput=False)
    WcT = nc.declare_dram_parameter("WcT", [I, G], F16, isOutput=False)
    WfaT = nc.declare_dram_parameter("WfaT", [H, F], F16, isOutput=False)
    WfcT = nc.declare_dram_parameter("WfcT", [F, NCLS], F16, isOutput=False)
    bias = nc.declare_dram_parameter("bias_total", [G], F32, isOutput=False)
    bfa = nc.declare_dram_parameter("bfa", [F], F32, isOutput=False)
    bfc = nc.declare_dram_parameter("bfc", [NCLS], F32, isOutput=False)
    ident = nc.declare_dram_parameter("ident32", [BS, BS], F16, isOutput=False)
    out = nc.declare_dram_parameter("out", [BS, NCLS], F32, isOutput=True)

    gx = nc.dram_tensor("gx", [t_steps, BS, G], F16)

    # K-major views of DRAM tensors: i = k*128 + p
    xT_r = xT[:].rearrange("(k p) t b -> p k (t b)", p=P)
    WiT_r = WiT[:].rearrange("(k p) g -> p k g", p=P)
    if not fp8_rec:
        WhT_r = WhT[:].rearrange("(k p) g -> p k g", p=P)
    WcT_r = WcT[:].rearrange("(k p) g -> p k g", p=P)
    WfaT_r = WfaT[:].rearrange("(k p) f -> p k f", p=P)
    WfcT_r = WfcT[:].rearrange("(q p) n -> p q n", p=P)
    vT_r = vT[:].rearrange("(k p) b -> p k b", p=P)
    gx_rows = gx[:].rearrange("t b g -> (t b) g")

    def bcast(src_ap, rows):
        # read a [cols] DRAM vector into [rows, cols] SBUF (partition bcast)
        return bass.AP(tensor=src_ap.tensor, offset=src_ap.offset,
                       ap=[[0, rows]] + list(src_ap.ap))

    def emit_once(tc, ctx):
        consts = ctx.enter_context(tc.tile_pool(name="consts", bufs=1))

        # ---- small resident constants ----
        bfa_rep = consts.tile([BS, F], F32, tag="bfa_rep")
        nc.sync.dma_start(out=bfa_rep, in_=bcast(bfa[:], BS))
        bfc_rep = consts.tile([BS, NCLS], F32, tag="bfc_rep")
        nc.sync.dma_start(out=bfc_rep, in_=bcast(bfc[:], BS))
        ident_sb = consts.tile([BS, BS], F16, tag="ident_sb")
        nc.sync.dma_start(out=ident_sb, in_=ident[:])

        with (
            tc.tile_pool(name="ph1_big", bufs=1) as ph1_big,
            tc.tile_pool(name="ph1_ps", bufs=4, space="PSUM") as ph1_ps,
            tc.tile_pool(name="ph1_x", bufs=2) as ph1_x,
            tc.tile_pool(name="ph1_out", bufs=3) as ph1_out,
        ):
            ctx4 = ph1_big.tile([P, G], F32, tag="ctx4")
            # fp8 recurrence pre-scales gx by 1024 (host also scales
            # bias_total); the 1/1024 compensation sits in the ACT drain.
            gx_scale = 1024.0 if fp8_rec else 1.0
            # ---------- phase 0: ctx4 = v @ WcT + biases, replicated 4x ----------
            with nc.named_scope("phase0_ctx"):
                with (
                    tc.tile_pool(name="ph0_w", bufs=2) as ph0_w,
                    tc.tile_pool(name="ph0_misc", bufs=1) as ph0_misc,
                ):
                    v_sb = ph0_misc.tile([P, KI, BS], F16, tag="v_sb")
                    nc.sync.dma_start(out=v_sb, in_=vT_r)
                    bias_rep = ph0_misc.tile([BS, G], F32, tag="bias_rep")
                    nc.sync.dma_start(out=bias_rep, in_=bcast(bias[:], BS))
                    for n in range(NCH):
                        wc_t = ph0_w.tile([P, KI, 512], F16, tag="wc")
                        nc.sync.dma_start(
                            out=wc_t, in_=WcT_r[:, :, 512 * n:512 * (n + 1)])
                        ps0 = ph1_ps.tile([BS, 512], F32, tag="ps0", bufs=2)
                        for k in range(KI):
                            nc.tensor.matmul(ps0, lhsT=v_sb[:, k, :],
                                             rhs=wc_t[:, k, :],
                                             start=(k == 0), stop=(k == KI - 1))
                        nc.vector.scalar_tensor_tensor(
                            out=ctx4[0:BS, 512 * n:512 * (n + 1)], in0=ps0,
                            scalar=gx_scale,
                            in1=bias_rep[:, 512 * n:512 * (n + 1)],
                            op0=AOP.mult, op1=AOP.add)
                    for r in range(1, 4):
                        nc.sync.dma_start(out=ctx4[BS * r:BS * (r + 1), :],
                                          in_=ctx4[0:BS, :])

            # ---------- phase 1: gx = x @ WiT + ctx4 ----------
            with nc.named_scope("phase1_gemm"):
                WiT_sb = ph1_big.tile([P, KI, G], F16, tag="WiT_sb")
                for k in range(KI):
                    nc.sync.dma_start(out=WiT_sb[:, k, :], in_=WiT_r[:, k, :])

                n_groups = (n_mtiles + 3) // 4
                for grp in range(n_groups):
                    mt_in_grp = min(4, n_mtiles - grp * 4)
                    x_sb = ph1_x.tile([P, KI, 512], F16, tag="x_sb")
                    nc.sync.dma_start(
                        out=x_sb[:, :, 0:128 * mt_in_grp],
                        in_=xT_r[:, :, 512 * grp:512 * grp + 128 * mt_in_grp])
                    for mi in range(mt_in_grp):
                        m = grp * 4 + mi
                        # batch the whole m-tile's output so the gx write is
                        # one fully-contiguous 1MB transfer (rows of [T,BS,G]
                        # are contiguous) instead of 8 strided 1KB-line DMAs.
                        gxo = ph1_out.tile([P, G], F16, tag="gxo")
                        for n in range(NCH):
                            ps1 = ph1_ps.tile([P, 512], F32, tag="ps1")
                            for k in range(KI):
                                nc.tensor.matmul(
                                    ps1, lhsT=x_sb[:, k, 128 * mi:128 * (mi + 1)],
                                    rhs=WiT_sb[:, k, 512 * n:512 * (n + 1)],
                                    start=(k == 0), stop=(k == KI - 1))
                            if ph1_no_drain:
                                continue
                            nc.vector.scalar_tensor_tensor(
                                out=gxo[:, 512 * n:512 * (n + 1)], in0=ps1,
                                scalar=gx_scale,
                                in1=ctx4[:, 512 * n:512 * (n + 1)],
                                op0=AOP.mult, op1=AOP.add)
                        if ph1_no_drain or ph1_no_write:
                            continue
                        ph1_wr = getattr(nc, ph1_wr_engine)
                        ph1_wr.dma_start(
                            out=gx_rows[128 * m:128 * (m + 1), :], in_=gxo)

        # ---------- phase 2: recurrence ----------
        with (
            tc.tile_pool(name="p2_w", bufs=1) as p2_w,
            tc.tile_pool(name="p2_state", bufs=1) as p2_state,
            tc.tile_pool(name="p2_gx", bufs=4) as p2_gx,
            tc.tile_pool(name="p2_act", bufs=10) as p2_act,
            tc.tile_pool(name="p2_cell", bufs=4) as p2_cell,
            tc.tile_pool(name="p2_ht", bufs=24) as p2_ht,
        ):
            if fp8_rec:
                Wh8_sb = p2_w.tile([P, KH // 2, 2, G], F8, tag="Wh8_sb")
                for dk in range(KH // 2):
                    nc.sync.dma_start(out=Wh8_sb[:, dk, :, :],
                                      in_=WhT8[:, dk, :, :])
            else:
                WhT_sb = p2_w.tile([P, KH, G], F16, tag="WhT_sb")
                for k in range(KH):
                    nc.sync.dma_start(out=WhT_sb[:, k, :], in_=WhT_r[:, k, :])
            WfaT_sb = p2_w.tile([P, KH, F], F16, tag="WfaT_sb")
            nc.sync.dma_start(out=WfaT_sb, in_=WfaT_r)
            WfcT_sb = p2_w.tile([P, F // P, NCLS], F16, tag="WfcT_sb")
            nc.sync.dma_start(out=WfcT_sb, in_=WfcT_r)

            c_st = p2_state.tile([BS, H], F32, tag="c_st")        # [32, 1024]
            nc.vector.memset(c_st, 0.0)
            hT = []
            hT8 = []
            if fp8_rec:
                for dk in range(4):
                    ht0 = p2_ht.tile([P, 2, BS], F16, tag="htq", name="ht0")
                    nc.vector.memset(ht0, 0.0)
                    hT.extend(ht0[:, i, :] for i in range(2))
                    ht80 = p2_ht.tile([P, 2, BS], F8, tag="ht8q", name="ht80")
                    nc.vector.memset(ht80, 0.0)
                    hT8.append(ht80[:, :, :])
            else:
                for j in range(2):
                    ht0 = p2_ht.tile([P, 4, BS], F16, tag="ht", name="ht0")
                    nc.vector.memset(ht0, 0.0)
                    hT.extend(ht0[:, q, :] for q in range(4))

            # chunk semantics: ig=0,1 fg=2,3 cg=4,5 og=6,7; half j uses
            # chunks {0+j, 2+j, 4+j, 6+j}. Per-half order ig -> cg -> fg -> og.
            # Each chunk: prologue (gx via identity-matmul into PSUM + k0..3)
            # then completion (k4..7) staggered so chunk completions -- and the
            # ACT drains that read PSUM directly -- spread across the step.
            ALLCH = [0, 4, 2, 6, 1, 5, 3, 7]
            with tc.tile_pool(name="p2_ps", bufs=8, space="PSUM") as p2_ps:
                for t in range(rec_steps):
                    with nc.named_scope("step"):
                        gx_sb = p2_gx.tile([BS, G], F16, tag="gx_sb")
                        nc.gpsimd.dma_start(out=gx_sb, in_=gx[t % t_steps, :, :])

                        ps_t = {}
                        pend = list(ALLCH)

                        def chunk_uses_ident(n):
                            # fp8: every chunk injects the (1024-prescaled) gx
                            # via identity-matmul and drains ACT-direct with
                            # scale=1/1024 -- shortest per-chunk tail. fp16:
                            # og chunks only (PE has no slack there).
                            if no_gx_mm:
                                return False
                            if fp8_rec or not gx_via_dve:
                                return True
                            return n in (6, 7)

                        # fp8 path: PSUM holds (16h)@(64Wh) + 1024*gx
                        drain_scale = (1.0 / 1024) if fp8_rec else 1.0

                        def prologue(n):
                            ps_t[n] = p2_ps.tile([BS, 512], F32, tag="ps2",
                                                 name="ps2")
                            if chunk_uses_ident(n):
                                nc.tensor.matmul(
                                    ps_t[n], lhsT=ident_sb,
                                    rhs=gx_sb[:, 512 * n:512 * (n + 1)],
                                    start=True, stop=False)
                            if fp8_rec:
                                for dk in range(2):
                                    nc.tensor.matmul(
                                        ps_t[n], lhsT=hT8[dk],
                                        rhs=Wh8_sb[:, dk, :,
                                                   512 * n:512 * (n + 1)],
                                        start=(not chunk_uses_ident(n)
                                               and dk == 0), stop=False,
                                        perf_mode=mybir.MatmulPerfMode.DoubleRow)
                            else:
                                for k in range(4):
                                    nc.tensor.matmul(
                                        ps_t[n], lhsT=hT[k],
                                        rhs=WhT_sb[:, k, 512 * n:512 * (n + 1)],
                                        start=(not chunk_uses_ident(n) and k == 0),
                                        stop=False)

                        def finish_chunk(n):
                            if fp8_rec:
                                for dk in range(2, KH // 2):
                                    nc.tensor.matmul(
                                        ps_t[n], lhsT=hT8[dk],
                                        rhs=Wh8_sb[:, dk, :,
                                                   512 * n:512 * (n + 1)],
                                        start=False, stop=(dk == KH // 2 - 1),
                                        perf_mode=mybir.MatmulPerfMode.DoubleRow)
                            else:
                                for k in range(4, KH):
                                    nc.tensor.matmul(
                                        ps_t[n], lhsT=hT[k],
                                        rhs=WhT_sb[:, k, 512 * n:512 * (n + 1)],
                                        start=False, stop=(k == KH - 1))
                            if pend:
                                prologue(pend.pop(0))
                            func = AFT.Tanh if n in (4, 5) else AFT.Sigmoid
                            ga = p2_act.tile([BS, 512], F16, tag="ga",
                                             name="ga")
                            if chunk_uses_ident(n) or no_gx_mm:
                                nc.scalar.activation(out=ga, in_=ps_t[n],
                                                     func=func,
                                                     scale=drain_scale)
                            else:
                                gs = p2_act.tile([BS, 512], F16, tag="gs",
                                                 name="gs")
                                nc.vector.scalar_tensor_tensor(
                                    out=gs, in0=ps_t[n], scalar=drain_scale,
                                    in1=gx_sb[:, 512 * n:512 * (n + 1)],
                                    op0=AOP.mult, op1=AOP.add)
                                nc.scalar.activation(out=gs if False else ga,
                                                     in_=gs, func=func)
                            return ga

                        for _ in range(3):
                            prologue(pend.pop(0))

                        hT_new = []
                        hT8_new = []
                        for j in range(2):        # unit halves 0:512, 512:1024
                            ch = c_st[:, 512 * j:512 * (j + 1)]
                            ig_a = finish_chunk(0 + j)
                            tcg = finish_chunk(4 + j)
                            t2 = p2_cell.tile([BS, 512], F16, tag="t2")
                            nc.vector.tensor_tensor(out=t2, in0=ig_a, in1=tcg,
                                                    op=AOP.mult)
                            fg_a = finish_chunk(2 + j)
                            t1 = p2_cell.tile([BS, 512], F32, tag="t1")
                            nc.vector.tensor_tensor(out=t1, in0=fg_a, in1=ch,
                                                    op=AOP.mult)
                            nc.vector.tensor_tensor(out=ch, in0=t1, in1=t2,
                                                    op=AOP.add)
                            tc_t = p2_cell.tile([BS, 512], F16, tag="tc_t")
                            nc.scalar.activation(out=tc_t, in_=ch, func=AFT.Tanh)
                            og_a = finish_chunk(6 + j)
                            if fp8_rec:
                                # h carried as 16*h (fp8 normal range; WfaT
                                # pre-scaled by 1/16 host-side). Produce in
                                # 256-col pieces so transpose+fp8-convert of
                                # dk=2j pipelines under piece 2j+1's ops and
                                # next-step matmuls start per-dk.
                                for q2 in range(2):
                                    sl = slice(256 * q2, 256 * (q2 + 1))
                                    hp = p2_cell.tile([BS, 256], F16,
                                                      tag="hp", name="hp")
                                    nc.vector.scalar_tensor_tensor(
                                        out=hp, in0=og_a[:, sl], scalar=16.0,
                                        in1=tc_t[:, sl],
                                        op0=AOP.mult, op1=AOP.mult)
                                    htq = p2_ht.tile([P, 2, BS], F16,
                                                     tag="htq", name="htq")
                                    nc.sync.dma_start_transpose(out=htq,
                                                                in_=hp)
                                    ht8q = p2_ht.tile([P, 2, BS], F8,
                                                      tag="ht8q", name="ht8q")
                                    nc.vector.tensor_scalar_mul(
                                        out=ht8q, in0=htq, scalar1=1.0)
                                    hT_new.extend(htq[:, i, :]
                                                  for i in range(2))
                                    hT8_new.append(ht8q[:, :, :])
                            else:
                                h_half = p2_cell.tile([BS, 512], F16,
                                                      tag="h_half")
                                nc.vector.tensor_tensor(out=h_half, in0=og_a,
                                                        in1=tc_t, op=AOP.mult)
                                # one xbar transpose: [32,512] -> [128,4,32]
                                # with [:, q, :] = h_T[512j+128q:..., :]
                                htn = p2_ht.tile([P, 4, BS], F16, tag="ht",
                                                 name="htn")
                                nc.sync.dma_start_transpose(out=htn, in_=h_half)
                                hT_new.extend(htn[:, q, :] for q in range(4))
                        hT = hT_new
                        if fp8_rec:
                            hT8 = hT8_new

            # ---------- head ----------
            with nc.named_scope("head"):
                with tc.tile_pool(name="head_ps", bufs=1, space="PSUM") as hps:
                    ps_f = hps.tile([BS, F], F32, tag="ps_f")
                    for k in range(KH):
                        nc.tensor.matmul(ps_f, lhsT=hT[k],
                                         rhs=WfaT_sb[:, k, :],
                                         start=(k == 0), stop=(k == KH - 1))
                    x1 = p2_cell.tile([BS, F], F32, tag="x1", bufs=1)
                    nc.vector.scalar_tensor_tensor(
                        out=x1, in0=ps_f, scalar=1.0, in1=bfa_rep,
                        op0=AOP.mult, op1=AOP.add)
                    x1r = p2_cell.tile([BS, F], F16, tag="x1r", bufs=1)
                    nc.scalar.activation(out=x1r, in_=x1, func=AFT.Relu)
                    x1T = p2_ht.tile([P, F // P, BS], F16, tag="x1T", bufs=1)
                    nc.sync.dma_start_transpose(out=x1T, in_=x1r)
                    ps_o = hps.tile([BS, NCLS], F32, tag="ps_o")
                    for q in range(F // P):
                        nc.tensor.matmul(ps_o, lhsT=x1T[:, q, :],
                                         rhs=WfcT_sb[:, q, :],
                                         start=(q == 0), stop=(q == F // P - 1))
                    out_sb = p2_cell.tile([BS, NCLS], F32, tag="out_sb", bufs=1)
                    nc.vector.scalar_tensor_tensor(
                        out=out_sb, in0=ps_o, scalar=1.0, in1=bfc_rep,
                        op0=AOP.mult, op1=AOP.add)
                    nc.sync.dma_start(out=out[:], in_=out_sb)

    with tile.TileContext(nc) as tc:
        for _rep in range(n_reps):
            with ExitStack() as ctx:
                emit_once(tc, ctx)

    nc.compile()
    return nc


def make_in_maps(inputs: dict, t_steps: int = T, fp8_rec: bool = False):
    """Shard + lay out the full inputs for the 8 cores (host-side numpy)."""
    x = np.asarray(inputs["i_features"], np.float32)[:, :t_steps, :]
    v = np.asarray(inputs["v_features"], np.float32)
    Wi, bi = np.asarray(inputs["Wi"], np.float32), np.asarray(inputs["bi"], np.float32)
    Wh, bh = np.asarray(inputs["Wh"], np.float32), np.asarray(inputs["bh"], np.float32)
    Wc, bc = np.asarray(inputs["Wc"], np.float32), np.asarray(inputs["bc"], np.float32)
    Wfa, bfa = np.asarray(inputs["Wfa"], np.float32), np.asarray(inputs["bfa"], np.float32)
    Wfc, bfc = np.asarray(inputs["Wfc"], np.float32), np.asarray(inputs["bfc"], np.float32)

    import ml_dtypes
    shared = {
        "WiT": np.ascontiguousarray(Wi.T).astype(np.float16),
        "WcT": np.ascontiguousarray(Wc.T).astype(np.float16),
        "WfcT": np.ascontiguousarray(Wfc.T).astype(np.float16),
        "bias_total": ((bi + bh + bc) * (1024.0 if fp8_rec else 1.0)).astype(np.float32),
        "bfa": bfa.astype(np.float32),
        "bfc": bfc.astype(np.float32),
        "ident32": np.eye(BS, dtype=np.float16),
    }
    if fp8_rec:
        # [p, dk, e, g] = 64*Wh.T[256dk+128e+p, g]; h carried as 16h, the
        # 1/1024 compensation folds into the PSUM drain; Wfa absorbs h's 16.
        wht = np.ascontiguousarray(Wh.T)                 # [H, G]
        wh8 = (64.0 * wht).reshape(H // 256, 2, P, G).transpose(2, 0, 1, 3)
        shared["WhT8"] = np.ascontiguousarray(wh8).astype(ml_dtypes.float8_e4m3)
        shared["WfaT"] = np.ascontiguousarray(Wfa.T / 16.0).astype(np.float16)
    else:
        shared["WhT"] = np.ascontiguousarray(Wh.T).astype(np.float16)
        shared["WfaT"] = np.ascontiguousarray(Wfa.T).astype(np.float16)
    in_maps = []
    nb = x.shape[0] // BS
    for s in range(nb):
        xs = x[s * BS:(s + 1) * BS]                      # [BS, t, I]
        in_maps.append({
            "xT": np.ascontiguousarray(xs.transpose(2, 1, 0)).astype(np.float16),
            "vT": np.ascontiguousarray(v[s * BS:(s + 1) * BS].T).astype(np.float16),
            **shared,
        })
    return in_maps


_NC_CACHE = {}


def kernel(**inputs) -> np.ndarray:
    in_maps = make_in_maps(inputs, T, fp8_rec=False)
    if T not in _NC_CACHE:
        _NC_CACHE[T] = build_lstm(T, fp8_rec=False)
    nc = _NC_CACHE[T]
    res = run_bass_kernel_spmd(nc, in_maps, core_ids=list(range(NCORES)))
    return np.concatenate([r["out"] for r in res.results], axis=0).astype(np.float32)

